# revision 19
# baseline (speedup 1.0000x reference)
"""Banded (sparse) multi-head attention block on 8 TRN2 NeuronCores.

Reference computation (B=4, N=1024, C=1024, H=16, D=64, epoch=25 -> band w=8):
    qkv = x @ Wqkv.T                      [B,N,3C], per-head interleaved split
    q,k,v per head; score = q k^T / sqrt(D); band mask |i-j|<=8; softmax
    ctx = attn @ v; out = ctx @ Wproj.T + bproj

Sharding: the band mask makes attention local, so we shard the sequence:
core = (b, s) with b in 0..3, s in 0..1 owns tokens [s*512, (s+1)*512) of
batch b plus an 8-token halo on each side.  No collectives are needed.

Per-core pipeline (tuned for the tensor engine):
  GEMM-V : v_nat[j, c]  with xt-stationary, dual-PSUM k-inner accumulation
  GEMM-QK: qk^T[c', n]  with w-stationary, dual-PSUM k-inner accumulation
  Attention per (head-pair, 128-row q-block, 144-wide window):
      band mask preloaded into PSUM as additive -1e9 bias (identity matmul),
      scores accumulate on top, exp on ACT with accum_out row-sums (=denoms),
      reciprocal+normalize on DVE, PE-transpose, ctx accumulated per-hp into
      one PSUM bank -> ctxT[hp] slabs [c, n]
  GEMM-O : out_nat[n, o] with ctxT-stationary, dual-PSUM k-inner, bias via
      DVE add during PSUM->SBUF eviction
A post-schedule pass removes back-to-back redundant LDWEIGHTS so paired
matmuls share one stationary load.
"""

import sys

if "/opt/trn_rl_repo" not in sys.path:
    sys.path.insert(0, "/opt/trn_rl_repo")

import numpy as np

B, N, C, H, D = 4, 1024, 1024, 16, 64
NO = 512          # owned tokens per core
HALO = 8
NL = NO + 2 * HALO    # 528 local tokens
WW = 144          # score window width per 128-row q block (128 main + 16 tail)
NBLK = 4          # q blocks of 128 per core
KT = 8            # contraction tiles (1024 / 128)
SCALE = D ** -0.5
NEG = -1.0e9

_CACHE = {}


def _dedup_ldweights(nc, mybir):
    """Remove InstLdweights whose stationary AP + flags match the previous
    ldweights on the PE stream with no intervening control flow.  Sync info
    on a removed duplicate is transferred to the next instruction."""
    removed = 0
    for fn in nc.m.functions:
        for blk in fn.blocks:
            insts = blk.instructions
            last_key = None
            drops = []
            for idx, inst in enumerate(insts):
                tname = type(inst).__name__
                if isinstance(inst, mybir.InstLdweights):
                    key = (
                        repr(inst.ins[0]),
                        getattr(inst, "is_transpose", None),
                        getattr(inst, "perf_mode", None),
                        getattr(inst, "tile_position", None),
                        getattr(inst, "tile_size", None),
                    )
                    if key == last_key:
                        drops.append((idx, inst))
                    else:
                        last_key = key
                elif isinstance(inst, mybir.InstMatmult):
                    pass  # does not change the loaded stationary
                elif "Branch" in tname or "ControlFlow" in tname or "Call" in tname:
                    last_key = None
            for idx, inst in drops:
                if inst.has_wait() or inst.has_update():
                    nxt = insts[idx + 1] if idx + 1 < len(insts) else None
                    if nxt is None:
                        continue
                    nxt.add_sync_dependencies_from(inst)
                insts.remove(inst)
                removed += 1
    return removed


def _build_nc():
    import concourse.bacc as bacc
    import concourse.tile as tile
    from concourse import mybir
    from concourse.masks import make_identity
    from contextlib import ExitStack

    f32 = mybir.dt.float32
    bf16 = mybir.dt.bfloat16
    EXP = mybir.ActivationFunctionType.Exp

    nc = bacc.Bacc(None, target_bir_lowering=False)

    xt_e = nc.declare_dram_parameter("xt", [128, KT * NL], bf16, isOutput=False)
    wqkb_e = nc.declare_dram_parameter("wqkb", [128, H * C], bf16, isOutput=False)
    wvt_e = nc.declare_dram_parameter("wvt", [128, KT * C], bf16, isOutput=False)
    wpt_e = nc.declare_dram_parameter("wpt", [128, KT * C], bf16, isOutput=False)
    bpb_e = nc.declare_dram_parameter("bpb", [128, C], bf16, isOutput=False)
    bias_e = nc.declare_dram_parameter("bias", [128, NBLK * 2 * WW], bf16,
                                       isOutput=False)
    out_e = nc.declare_dram_parameter("out", [NO, C], f32, isOutput=True)

    with tile.TileContext(nc) as tc, ExitStack() as ctx:
        const = ctx.enter_context(tc.tile_pool(name="const", bufs=1))
        xts = ctx.enter_context(tc.tile_pool(name="xts", bufs=1))
        wv_pool = ctx.enter_context(tc.tile_pool(name="wvp", bufs=1))
        wqk_pool = ctx.enter_context(tc.tile_pool(name="wqkp", bufs=1))
        wpt_pool = ctx.enter_context(tc.tile_pool(name="wptp", bufs=1))
        qk_pool = ctx.enter_context(tc.tile_pool(name="qksb", bufs=1))
        v_pool = ctx.enter_context(tc.tile_pool(name="vsb", bufs=1))
        ctx_pool = ctx.enter_context(tc.tile_pool(name="ctxsb", bufs=1))
        ex_pool = ctx.enter_context(tc.tile_pool(name="exp", bufs=4))
        at_pool = ctx.enter_context(tc.tile_pool(name="atp", bufs=8))
        atT_pool = ctx.enter_context(tc.tile_pool(name="atTp", bufs=2))
        dn_pool = ctx.enter_context(tc.tile_pool(name="dnp", bufs=4))
        out_pool = ctx.enter_context(tc.tile_pool(name="outp", bufs=3))

        class _View:
            """Column-window view into a wide [128, X] tile; supports
            v[sp, sc] with explicit 2D slices like a real tile."""
            def __init__(self, t, base):
                self.t, self.base = t, base
            def __getitem__(self, idx):
                sp, sc = idx
                return self.t[sp, self.base + sc.start:self.base + sc.stop]

        # ---- consolidated DMAs (queue order: qk0/qk1 feed, xt, wv, ...) --
        wqk_all = wqk_pool.tile([128, H * C], bf16, tag="wqk")
        wqk_t = [_View(wqk_all, g * C) for g in range(H)]
        nc.sync.dma_start(out=wqk_all[:, 0:2 * C], in_=wqkb_e[:, 0:2 * C])
        xt_all = xts.tile([128, KT * NL], bf16, tag="xt")
        xt_t = [_View(xt_all, k * NL) for k in range(KT)]
        nc.sync.dma_start(out=xt_all[:], in_=xt_e[:])
        wv_all = wv_pool.tile([128, KT * C], bf16, tag="wv")
        wv_t = [_View(wv_all, k * C) for k in range(KT)]
        nc.sync.dma_start(out=wv_all[:, 0:4 * C], in_=wvt_e[:, 0:4 * C])
        nc.sync.dma_start(out=wv_all[:, 4 * C:KT * C], in_=wvt_e[:, 4 * C:KT * C])
        bias_sb = const.tile([128, NBLK * 2 * WW], bf16, tag="bias")
        nc.sync.dma_start(out=bias_sb[:], in_=bias_e[:])
        nc.sync.dma_start(out=wqk_all[:, 2 * C:H * C], in_=wqkb_e[:, 2 * C:H * C])
        wpt_all = wpt_pool.tile([128, KT * C], bf16, tag="wpt")
        wpt_t = [_View(wpt_all, k * C) for k in range(KT)]
        nc.sync.dma_start(out=wpt_all[:], in_=wpt_e[:])
        bpb_sb = const.tile([128, C], bf16, tag="bpb")
        nc.sync.dma_start(out=bpb_sb[:], in_=bpb_e[:])

        ident = const.tile([128, 128], bf16, tag="ident")
        make_identity(nc, ident[:])

        # persistent activation slabs; kxa = [k_even | 0], kxb = [0 | k_odd]
        # so score matmuls contract the full 128 partitions at base 0.
        q_sb, kxa_sb, kxb_sb = [], [], []
        for hp in range(8):
            tq = qk_pool.tile([128, NL], bf16, tag=f"q{hp}")
            q_sb.append(tq)
            ta = qk_pool.tile([128, NL], bf16, tag=f"kxa{hp}")
            nc.vector.memset(ta[64:128, :], 0.0)
            kxa_sb.append(ta)
            tb = qk_pool.tile([128, NL], bf16, tag=f"kxb{hp}")
            nc.vector.memset(tb[0:64, :], 0.0)
            kxb_sb.append(tb)
        v_sb = []
        for j in range(5):
            t = v_pool.tile([128, C], bf16, tag=f"v{j}")
            v_sb.append(t)
        ctxT = []
        for cb in range(8):
            t = ctx_pool.tile([128, NO], bf16, tag=f"ctx{cb}")
            ctxT.append(t)

        def emit_qk_with(g, ps0, ps1, copy):
            wt = wqk_t[g]
            for k in range(KT):
                nc.tensor.matmul(
                    ps0[:128, 0:264],
                    lhsT=wt[:, k * 128:(k + 1) * 128],
                    rhs=xt_t[k][:, 0:264],
                    start=(k == 0), stop=(k == KT - 1),
                )
                nc.tensor.matmul(
                    ps1[:128, 0:264],
                    lhsT=wt[:, k * 128:(k + 1) * 128],
                    rhs=xt_t[k][:, 264:528],
                    start=(k == 0), stop=(k == KT - 1),
                )
            copy(g, ps0, ps1)

        def qk_copies(g, ps0, ps1):
            if g % 2 == 0:
                nc.scalar.copy(out=q_sb[g // 2][:, 0:264], in_=ps0[:128, 0:264])
                nc.vector.tensor_copy(out=q_sb[g // 2][:, 264:528],
                                      in_=ps1[:128, 0:264])
            else:
                nc.scalar.copy(
                    out=kxa_sb[g // 2][0:64, 0:264], in_=ps0[0:64, 0:264])
                nc.vector.tensor_copy(
                    out=kxb_sb[g // 2][64:128, 0:264], in_=ps0[64:128, 0:264])
                nc.scalar.copy(
                    out=kxa_sb[g // 2][0:64, 264:528], in_=ps1[0:64, 0:264])
                nc.vector.tensor_copy(
                    out=kxb_sb[g // 2][64:128, 264:528], in_=ps1[64:128, 0:264])

        # ---- phase V (prefixed by qk0/qk1 warm-up while wv streams) ----
        with tc.tile_pool(name="psv", bufs=2, space="PSUM") as psv:
            for g in (0, 1):
                p0 = psv.tile([128, 512], f32, tag="psv0")
                p1 = psv.tile([128, 512], f32, tag="psv1")
                emit_qk_with(g, p0, p1, qk_copies)
            for jb in range(5):
                pj = 128 if jb < 4 else 16
                ps0 = psv.tile([128, 512], f32, tag="psv0")
                ps1 = psv.tile([128, 512], f32, tag="psv1")
                for k in range(KT):
                    nc.tensor.matmul(
                        ps0[:pj, :],
                        lhsT=xt_t[k][:, jb * 128:jb * 128 + pj],
                        rhs=wv_t[k][:, 0:512],
                        start=(k == 0), stop=(k == KT - 1),
                    )
                    nc.tensor.matmul(
                        ps1[:pj, :],
                        lhsT=xt_t[k][:, jb * 128:jb * 128 + pj],
                        rhs=wv_t[k][:, 512:1024],
                        start=(k == 0), stop=(k == KT - 1),
                    )
                nc.scalar.copy(out=v_sb[jb][:pj, 0:512], in_=ps0[:pj, :])
                nc.scalar.copy(out=v_sb[jb][:pj, 512:1024], in_=ps1[:pj, :])

        # ---- main: QK GEMM interleaved with attention ------------------
        with tc.tile_pool(name="psqk", bufs=2, space="PSUM") as psqk, \
             tc.tile_pool(name="pss", bufs=2, space="PSUM") as pss_pool, \
             tc.tile_pool(name="pst", bufs=1, space="PSUM") as pst_pool, \
             tc.tile_pool(name="psc", bufs=1, space="PSUM") as psc_pool:
            state = {}

            def emit_qk(g):
                ps0 = psqk.tile([128, 264], f32, tag="psqk0")
                ps1 = psqk.tile([128, 264], f32, tag="psqk1")
                emit_qk_with(g, ps0, ps1, qk_copies)

            def emit_scores(hp, blk):
                j0 = blk * 128
                q0 = HALO + blk * 128
                b0 = blk * 2 * WW
                ps = pss_pool.tile([128, 2 * WW], f32, tag="pss")
                nc.tensor.matmul(
                    ps[:, 0:WW], lhsT=ident[:, 0:128],
                    rhs=bias_sb[:, b0:b0 + WW], start=True, stop=False)
                nc.tensor.matmul(
                    ps[:, 0:WW], lhsT=q_sb[hp][:, q0:q0 + 128],
                    rhs=kxa_sb[hp][:, j0:j0 + WW], start=False, stop=True)
                nc.tensor.matmul(
                    ps[:, WW:2 * WW], lhsT=ident[:, 0:128],
                    rhs=bias_sb[:, b0 + WW:b0 + 2 * WW], start=True, stop=False)
                nc.tensor.matmul(
                    ps[:, WW:2 * WW], lhsT=q_sb[hp][:, q0:q0 + 128],
                    rhs=kxb_sb[hp][:, j0:j0 + WW], start=False, stop=True)
                ex = ex_pool.tile([128, 2 * WW], bf16, tag="ex")
                den = dn_pool.tile([128, 4], f32, tag="den")
                for hi in range(2):
                    nc.scalar.activation(
                        out=ex[:, hi * WW:(hi + 1) * WW],
                        in_=ps[:, hi * WW:(hi + 1) * WW],
                        func=EXP, accum_out=den[:, hi:hi + 1])
                nc.vector.reciprocal(out=den[:, 2:4], in_=den[:, 0:2])
                at = at_pool.tile([128, 2 * WW], bf16, tag="at")
                for hi in range(2):
                    nc.vector.tensor_scalar_mul(
                        out=at[:, hi * WW:(hi + 1) * WW],
                        in0=ex[:, hi * WW:(hi + 1) * WW],
                        scalar1=den[:, 2 + hi:3 + hi])
                state[(hp, blk)] = at

            def emit_attnout(hp):
                pc = psc_pool.tile([128, NO], f32, tag="psc")
                for blk in range(NBLK):
                    at = state.pop((hp, blk))
                    # pt: [0:128]=hA main^T, [128:256]=hB main^T,
                    #     [0:16, 256:384]=hA tail^T, [0:16, 384:512]=hB tail^T
                    pt = pst_pool.tile([128, 512], bf16, tag="pt")
                    nc.tensor.transpose(pt[:, 0:128], at[:, 0:128], ident[:])
                    nc.tensor.transpose(pt[:, 128:256], at[:, WW:WW + 128],
                                        ident[:])
                    nc.tensor.transpose(pt[0:16, 256:384], at[:, 128:WW],
                                        ident[:])
                    nc.tensor.transpose(pt[0:16, 384:512], at[:, WW + 128:2 * WW],
                                        ident[:])
                    atT = atT_pool.tile([128, 512], bf16, tag="atT")
                    nc.vector.tensor_copy(out=atT[:, 0:256], in_=pt[:, 0:256])
                    nc.vector.tensor_copy(out=atT[0:16, 256:512],
                                          in_=pt[0:16, 256:512])
                    for hi in range(2):
                        h = 2 * hp + hi
                        nc.tensor.matmul(
                            pc[hi * 64:(hi + 1) * 64, blk * 128:(blk + 1) * 128],
                            lhsT=v_sb[blk][:, h * 64:(h + 1) * 64],
                            rhs=atT[:, hi * 128:hi * 128 + 128],
                            start=True, stop=False,
                        )
                        nc.tensor.matmul(
                            pc[hi * 64:(hi + 1) * 64, blk * 128:(blk + 1) * 128],
                            lhsT=v_sb[blk + 1][0:16, h * 64:(h + 1) * 64],
                            rhs=atT[0:16, 256 + hi * 128:256 + hi * 128 + 128],
                            start=False, stop=True,
                        )
                nc.scalar.copy(out=ctxT[hp][:], in_=pc[:])

            for hp in range(8):
                emit_scores(hp, 0)
                emit_scores(hp, 1)
                if 2 * hp + 2 < H:
                    emit_qk(2 * hp + 2)
                emit_scores(hp, 2)
                emit_scores(hp, 3)
                if 2 * hp + 3 < H:
                    emit_qk(2 * hp + 3)
                if hp >= 1:
                    emit_attnout(hp - 1)
            emit_attnout(7)

        # ---- phase O: out_nat[n, o] (ctxT-stationary, k-inner) ---------
        with tc.tile_pool(name="pso", bufs=2, space="PSUM") as pso:
            for t in range(4):
                ps0 = pso.tile([128, 512], f32, tag="pso0")
                ps1 = pso.tile([128, 512], f32, tag="pso1")
                for k in range(KT):
                    nc.tensor.matmul(
                        ps0[:],
                        lhsT=ctxT[k][:, t * 128:(t + 1) * 128],
                        rhs=wpt_t[k][:, 0:512],
                        start=(k == 0), stop=(k == KT - 1),
                    )
                    nc.tensor.matmul(
                        ps1[:],
                        lhsT=ctxT[k][:, t * 128:(t + 1) * 128],
                        rhs=wpt_t[k][:, 512:1024],
                        start=(k == 0), stop=(k == KT - 1),
                    )
                for oh, psx in ((0, ps0), (1, ps1)):
                    ot = out_pool.tile([128, 512], f32, tag="ot")
                    nc.vector.tensor_add(
                        out=ot[:], in0=psx[:],
                        in1=bpb_sb[:, oh * 512:(oh + 1) * 512])
                    nc.sync.dma_start(
                        out=out_e[t * 128:(t + 1) * 128, oh * 512:(oh + 1) * 512],
                        in_=ot[:])

    from concourse import mybir as _mb
    import os as _os
    if _os.environ.get("KERNEL_NO_DEDUP"):
        n = 0
    else:
        n = _dedup_ldweights(nc, _mb)
    _CACHE["ldweights_removed"] = n
    nc.compile()
    return nc


def _get_nc():
    if "nc" not in _CACHE:
        _CACHE["nc"] = _build_nc()
    return _CACHE["nc"]


def _band_width(epoch):
    if epoch is None or epoch >= 50:
        return None
    if epoch < 20:
        return 6
    if epoch < 30:
        return 8
    if epoch < 40:
        return 10
    return 12


def _numpy_ref(x, Wqkv, Wproj, bproj, w):
    """Pure-numpy fallback for band widths this kernel wasn't compiled for."""
    b, n, c = x.shape
    d = c // H
    qkv = np.einsum("bnc,oc->bno", x, Wqkv)
    qkv = qkv.reshape(b, n, H, 3 * d).transpose(0, 2, 1, 3)
    q, k, v = np.split(qkv, 3, axis=-1)
    score = np.einsum("bhid,bhjd->bhij", q, k) * (d ** -0.5)
    if w is not None:
        idx = np.arange(n)
        mask = np.abs(idx[:, None] - idx[None, :]) <= w
        score = np.where(mask[None, None], score, np.float32(-1e9))
    score -= score.max(axis=-1, keepdims=True)
    e = np.exp(score)
    attn = e / e.sum(axis=-1, keepdims=True)
    ctxv = np.einsum("bhij,bhjd->bhid", attn, v)
    ctxv = ctxv.transpose(0, 2, 1, 3).reshape(b, n, c)
    return (np.einsum("bnc,oc->bno", ctxv, Wproj) + bproj).astype(np.float32)


def _prep_in_maps(x, Wqkv, Wproj, bproj):
    import ml_dtypes
    bf = ml_dtypes.bfloat16
    x = np.ascontiguousarray(np.asarray(x, dtype=np.float32))
    Wqkv = np.asarray(Wqkv, dtype=np.float32)
    Wproj = np.asarray(Wproj, dtype=np.float32)
    bproj = np.asarray(bproj, dtype=np.float32)

    # qk weight output-blocks g: even g -> [q_{2hp} | q_{2hp+1}] (prescaled),
    # odd g -> [k_{2hp} | k_{2hp+1}]
    wsplit = Wqkv.reshape(H, 3, D, C)
    wq = wsplit[:, 0] * np.float32(SCALE)                      # [H, D, C]
    wk = wsplit[:, 1]                                          # [H, D, C]
    wv = wsplit[:, 2]                                          # [H, D, C]
    wg = np.empty((H, 128, C), dtype=np.float32)
    wg[0::2] = wq.reshape(8, 128, C)
    wg[1::2] = wk.reshape(8, 128, C)
    # wqkb[g, p, k*128+m] = wg[g, m, k*128+p]: per-g contiguous [128, C]
    # slabs whose col-block k is the k-th contraction tile's lhsT
    wqkb = (wg.transpose(0, 2, 1).reshape(H, KT, 128, 128).transpose(0, 2, 1, 3)
            .reshape(H, 128, C))
    wqkb = np.ascontiguousarray(
        wqkb.transpose(1, 0, 2).reshape(128, H * C)).astype(bf)
    wvt = wv.reshape(H * D, C).T                               # [C, C]
    wvt = np.ascontiguousarray(
        wvt.reshape(KT, 128, C).transpose(1, 0, 2).reshape(128, KT * C)).astype(bf)
    wpt = np.ascontiguousarray(
        Wproj.T.reshape(KT, 128, C).transpose(1, 0, 2)
        .reshape(128, KT * C)).astype(bf)
    bpb = np.ascontiguousarray(
        np.broadcast_to(bproj[None, :], (128, C))).astype(bf)

    # additive score bias per sequence-half s: 0 where in-band and the k
    # column is a real token, else -1e9.  Layout [128, blk*(2*WW)] with the
    # per-block [128, WW] pattern duplicated for the two heads of a pair.
    r = np.arange(128)[:, None]
    jj = np.arange(WW)[None, :]
    band = (jj >= r) & (jj <= r + 2 * HALO)                    # [128, WW]
    biases = []
    for s in (0, 1):
        m = np.full((128, NBLK * 2 * WW), NEG, dtype=np.float32)
        for blk in range(NBLK):
            mloc = blk * 128 + jj                              # local k index
            valid = (mloc >= HALO) if s == 0 else (mloc < NO + HALO)
            bb = np.where(band & valid, 0.0, NEG).astype(np.float32)
            m[:, blk * 2 * WW:blk * 2 * WW + WW] = bb
            m[:, blk * 2 * WW + WW:(blk + 1) * 2 * WW] = bb
        biases.append(m.astype(bf))

    in_maps = []
    for core in range(8):
        b, s = core // 2, core % 2
        xloc = np.zeros((NL, C), dtype=np.float32)
        g0 = s * NO - HALO
        lo, hi = max(0, g0), min(N, g0 + NL)
        xloc[lo - g0:hi - g0] = x[b, lo:hi]
        xtp = np.ascontiguousarray(
            xloc.T.reshape(KT, 128, NL).transpose(1, 0, 2)
            .reshape(128, KT * NL)).astype(bf)
        in_maps.append({
            "xt": xtp,
            "wqkb": wqkb, "wvt": wvt, "wpt": wpt, "bpb": bpb,
            "bias": biases[s],
        })
    return in_maps


def kernel(x, Wqkv, Wproj, bproj, epoch):
    ep = None if epoch is None else int(np.asarray(epoch))
    w = _band_width(ep)
    if w != HALO:
        return _numpy_ref(np.asarray(x, np.float32), np.asarray(Wqkv, np.float32),
                          np.asarray(Wproj, np.float32),
                          np.asarray(bproj, np.float32), w)

    from concourse.bass_utils import run_bass_kernel_spmd

    nc = _get_nc()
    in_maps = _prep_in_maps(x, Wqkv, Wproj, bproj)
    res = run_bass_kernel_spmd(nc, in_maps, core_ids=list(range(8)))
    _CACHE["last_results"] = res

    out = np.empty((B, N, C), dtype=np.float32)
    for core in range(8):
        b, s = core // 2, core % 2
        out[b, s * NO:(s + 1) * NO, :] = res.results[core]["out"]
    return out


# revision 21
# speedup vs baseline: 1.1824x; 1.1824x over previous
"""Banded (sparse) multi-head attention block on 8 TRN2 NeuronCores.

Reference computation (B=4, N=1024, C=1024, H=16, D=64, epoch=25 -> band w=8):
    qkv = x @ Wqkv.T                      [B,N,3C], per-head interleaved split
    q,k,v per head; score = q k^T / sqrt(D); band mask |i-j|<=8; softmax
    ctx = attn @ v; out = ctx @ Wproj.T + bproj

Sharding: the band mask makes attention local, so we shard the sequence:
core = (b, s) with b in 0..3, s in 0..1 owns tokens [s*512, (s+1)*512) of
batch b plus an 8-token halo on each side.  No collectives are needed.

Per-core pipeline (tuned for the tensor engine):
  GEMM-V : v_nat[j, c]  with xt-stationary, dual-PSUM k-inner accumulation
  GEMM-QK: qk^T[c', n]  with w-stationary, dual-PSUM k-inner accumulation
  Attention per (head-pair, 128-row q-block, 144-wide window):
      band mask preloaded into PSUM as additive -1e9 bias (identity matmul),
      scores accumulate on top, exp on ACT with accum_out row-sums (=denoms),
      reciprocal+normalize on DVE, PE-transpose, ctx accumulated per-hp into
      one PSUM bank -> ctxT[hp] slabs [c, n]
  GEMM-O : out_nat[n, o] with ctxT-stationary, dual-PSUM k-inner, bias via
      DVE add during PSUM->SBUF eviction
A post-schedule pass removes back-to-back redundant LDWEIGHTS so paired
matmuls share one stationary load.
"""

import sys

if "/opt/trn_rl_repo" not in sys.path:
    sys.path.insert(0, "/opt/trn_rl_repo")

import numpy as np

B, N, C, H, D = 4, 1024, 1024, 16, 64
NO = 512          # owned tokens per core
HALO = 8
NL = NO + 2 * HALO    # 528 local tokens
WW = 144          # score window width per 128-row q block (128 main + 16 tail)
NBLK = 4          # q blocks of 128 per core
KT = 8            # contraction tiles (1024 / 128)
SCALE = D ** -0.5
NEG = -1.0e9

_CACHE = {}


def _dedup_ldweights(nc, mybir):
    """Remove InstLdweights whose stationary AP + flags match the previous
    ldweights on the PE stream with no intervening control flow.  Sync info
    on a removed duplicate is transferred to the next instruction."""
    removed = 0
    for fn in nc.m.functions:
        for blk in fn.blocks:
            insts = blk.instructions
            last_key = None
            drops = []
            for idx, inst in enumerate(insts):
                tname = type(inst).__name__
                if isinstance(inst, mybir.InstLdweights):
                    key = (
                        repr(inst.ins[0]),
                        getattr(inst, "is_transpose", None),
                        getattr(inst, "perf_mode", None),
                        getattr(inst, "tile_position", None),
                        getattr(inst, "tile_size", None),
                    )
                    if key == last_key:
                        drops.append((idx, inst))
                    else:
                        last_key = key
                elif isinstance(inst, mybir.InstMatmult):
                    pass  # does not change the loaded stationary
                elif "Branch" in tname or "ControlFlow" in tname or "Call" in tname:
                    last_key = None
            for idx, inst in drops:
                if inst.has_wait() or inst.has_update():
                    nxt = insts[idx + 1] if idx + 1 < len(insts) else None
                    if nxt is None:
                        continue
                    nxt.add_sync_dependencies_from(inst)
                insts.remove(inst)
                removed += 1
    return removed


def _build_nc():
    import concourse.bacc as bacc
    import concourse.tile as tile
    from concourse import mybir
    from concourse.masks import make_identity
    from contextlib import ExitStack

    f32 = mybir.dt.float32
    bf16 = mybir.dt.bfloat16
    EXP = mybir.ActivationFunctionType.Exp

    nc = bacc.Bacc(None, target_bir_lowering=False)

    xt_e = nc.declare_dram_parameter("xt", [128, KT * NL], bf16, isOutput=False)
    wqkb_e = nc.declare_dram_parameter("wqkb", [128, H * C], bf16, isOutput=False)
    wvt_e = nc.declare_dram_parameter("wvt", [128, KT * C], bf16, isOutput=False)
    wpt_e = nc.declare_dram_parameter("wpt", [128, KT * C], bf16, isOutput=False)
    bpb_e = nc.declare_dram_parameter("bpb", [128, C], bf16, isOutput=False)
    bias_e = nc.declare_dram_parameter("bias", [128, NBLK * 2 * WW], bf16,
                                       isOutput=False)
    out_e = nc.declare_dram_parameter("out", [NO, C], f32, isOutput=True)

    with tile.TileContext(nc) as tc, ExitStack() as ctx:
        const = ctx.enter_context(tc.tile_pool(name="const", bufs=1))
        xts = ctx.enter_context(tc.tile_pool(name="xts", bufs=1))
        wv_pool = ctx.enter_context(tc.tile_pool(name="wvp", bufs=1))
        wqk_pool = ctx.enter_context(tc.tile_pool(name="wqkp", bufs=1))
        wpt_pool = ctx.enter_context(tc.tile_pool(name="wptp", bufs=1))
        qk_pool = ctx.enter_context(tc.tile_pool(name="qksb", bufs=1))
        v_pool = ctx.enter_context(tc.tile_pool(name="vsb", bufs=1))
        ctx_pool = ctx.enter_context(tc.tile_pool(name="ctxsb", bufs=1))
        ex_pool = ctx.enter_context(tc.tile_pool(name="exp", bufs=4))
        at_pool = ctx.enter_context(tc.tile_pool(name="atp", bufs=8))
        atT_pool = ctx.enter_context(tc.tile_pool(name="atTp", bufs=2))
        dn_pool = ctx.enter_context(tc.tile_pool(name="dnp", bufs=4))
        out_pool = ctx.enter_context(tc.tile_pool(name="outp", bufs=3))

        class _View:
            """Column-window view into a wide [128, X] tile; supports
            v[sp, sc] with explicit 2D slices like a real tile."""
            def __init__(self, t, base):
                self.t, self.base = t, base
            def __getitem__(self, idx):
                sp, sc = idx
                return self.t[sp, self.base + sc.start:self.base + sc.stop]

        # ---- DMAs, pairwise-chunked: half the fixed costs of per-tile
        # transfers while consumers wait only on 2-tile windows ----------
        wqk_all = wqk_pool.tile([128, H * C], bf16, tag="wqk")
        wqk_t = [_View(wqk_all, g * C) for g in range(H)]
        nc.sync.dma_start(out=wqk_all[:, 0:2 * C], in_=wqkb_e[:, 0:2 * C])
        xt_all = xts.tile([128, KT * NL], bf16, tag="xt")
        xt_t = [_View(xt_all, k * NL) for k in range(KT)]
        for c0 in range(0, KT, 2):
            nc.sync.dma_start(
                out=xt_all[:, c0 * NL:(c0 + 2) * NL],
                in_=xt_e[:, c0 * NL:(c0 + 2) * NL])
        bias_sb = const.tile([128, NBLK * 2 * WW], bf16, tag="bias")
        nc.sync.dma_start(out=bias_sb[:], in_=bias_e[:])
        wv_all = wv_pool.tile([128, KT * C], bf16, tag="wv")
        wv_t = [_View(wv_all, k * C) for k in range(KT)]
        for c0 in range(0, KT, 2):
            nc.sync.dma_start(
                out=wv_all[:, c0 * C:(c0 + 2) * C],
                in_=wvt_e[:, c0 * C:(c0 + 2) * C])
        for c0 in range(2, H, 2):
            nc.sync.dma_start(
                out=wqk_all[:, c0 * C:(c0 + 2) * C],
                in_=wqkb_e[:, c0 * C:(c0 + 2) * C])
        wpt_all = wpt_pool.tile([128, KT * C], bf16, tag="wpt")
        wpt_t = [_View(wpt_all, k * C) for k in range(KT)]
        for c0 in range(0, KT, 4):
            nc.sync.dma_start(
                out=wpt_all[:, c0 * C:(c0 + 4) * C],
                in_=wpt_e[:, c0 * C:(c0 + 4) * C])
        bpb_sb = const.tile([128, C], bf16, tag="bpb")
        nc.sync.dma_start(out=bpb_sb[:], in_=bpb_e[:])

        ident = const.tile([128, 128], bf16, tag="ident")
        make_identity(nc, ident[:])

        # persistent activation slabs; kxa = [k_even | 0], kxb = [0 | k_odd]
        # so score matmuls contract the full 128 partitions at base 0.
        q_sb, kxa_sb, kxb_sb = [], [], []
        for hp in range(8):
            tq = qk_pool.tile([128, NL], bf16, tag=f"q{hp}")
            q_sb.append(tq)
            ta = qk_pool.tile([128, NL], bf16, tag=f"kxa{hp}")
            nc.vector.memset(ta[64:128, :], 0.0)
            kxa_sb.append(ta)
            tb = qk_pool.tile([128, NL], bf16, tag=f"kxb{hp}")
            nc.vector.memset(tb[0:64, :], 0.0)
            kxb_sb.append(tb)
        v_sb = []
        for j in range(5):
            t = v_pool.tile([128, C], bf16, tag=f"v{j}")
            v_sb.append(t)
        ctxT = []
        for cb in range(8):
            t = ctx_pool.tile([128, NO], bf16, tag=f"ctx{cb}")
            ctxT.append(t)

        def emit_qk_with(g, ps0, ps1, copy):
            wt = wqk_t[g]
            for k in range(KT):
                nc.tensor.matmul(
                    ps0[:128, 0:264],
                    lhsT=wt[:, k * 128:(k + 1) * 128],
                    rhs=xt_t[k][:, 0:264],
                    start=(k == 0), stop=(k == KT - 1),
                )
                nc.tensor.matmul(
                    ps1[:128, 0:264],
                    lhsT=wt[:, k * 128:(k + 1) * 128],
                    rhs=xt_t[k][:, 264:528],
                    start=(k == 0), stop=(k == KT - 1),
                )
            copy(g, ps0, ps1)

        def qk_copies(g, ps0, ps1):
            if g % 2 == 0:
                nc.scalar.copy(out=q_sb[g // 2][:, 0:264], in_=ps0[:128, 0:264])
                nc.vector.tensor_copy(out=q_sb[g // 2][:, 264:528],
                                      in_=ps1[:128, 0:264])
            else:
                nc.scalar.copy(
                    out=kxa_sb[g // 2][0:64, 0:264], in_=ps0[0:64, 0:264])
                nc.vector.tensor_copy(
                    out=kxb_sb[g // 2][64:128, 0:264], in_=ps0[64:128, 0:264])
                nc.scalar.copy(
                    out=kxa_sb[g // 2][0:64, 264:528], in_=ps1[0:64, 0:264])
                nc.vector.tensor_copy(
                    out=kxb_sb[g // 2][64:128, 264:528], in_=ps1[64:128, 0:264])

        # ---- phase V (prefixed by qk0/qk1 warm-up while wv streams) ----
        with tc.tile_pool(name="psv", bufs=2, space="PSUM") as psv:
            for g in (0, 1):
                p0 = psv.tile([128, 512], f32, tag="psv0")
                p1 = psv.tile([128, 512], f32, tag="psv1")
                emit_qk_with(g, p0, p1, qk_copies)
            for jb in range(5):
                pj = 128 if jb < 4 else 16
                ps0 = psv.tile([128, 512], f32, tag="psv0")
                ps1 = psv.tile([128, 512], f32, tag="psv1")
                for k in range(KT):
                    nc.tensor.matmul(
                        ps0[:pj, :],
                        lhsT=xt_t[k][:, jb * 128:jb * 128 + pj],
                        rhs=wv_t[k][:, 0:512],
                        start=(k == 0), stop=(k == KT - 1),
                    )
                    nc.tensor.matmul(
                        ps1[:pj, :],
                        lhsT=xt_t[k][:, jb * 128:jb * 128 + pj],
                        rhs=wv_t[k][:, 512:1024],
                        start=(k == 0), stop=(k == KT - 1),
                    )
                nc.scalar.copy(out=v_sb[jb][:pj, 0:512], in_=ps0[:pj, :])
                nc.scalar.copy(out=v_sb[jb][:pj, 512:1024], in_=ps1[:pj, :])

        # ---- main: QK GEMM interleaved with attention ------------------
        with tc.tile_pool(name="psqk", bufs=2, space="PSUM") as psqk, \
             tc.tile_pool(name="pss", bufs=2, space="PSUM") as pss_pool, \
             tc.tile_pool(name="pst", bufs=1, space="PSUM") as pst_pool, \
             tc.tile_pool(name="psc", bufs=1, space="PSUM") as psc_pool:
            state = {}

            def emit_qk(g):
                ps0 = psqk.tile([128, 264], f32, tag="psqk0")
                ps1 = psqk.tile([128, 264], f32, tag="psqk1")
                emit_qk_with(g, ps0, ps1, qk_copies)

            def emit_scores(hp, blk):
                j0 = blk * 128
                q0 = HALO + blk * 128
                b0 = blk * 2 * WW
                ps = pss_pool.tile([128, 2 * WW], f32, tag="pss")
                nc.tensor.matmul(
                    ps[:, 0:WW], lhsT=ident[:, 0:128],
                    rhs=bias_sb[:, b0:b0 + WW], start=True, stop=False)
                nc.tensor.matmul(
                    ps[:, 0:WW], lhsT=q_sb[hp][:, q0:q0 + 128],
                    rhs=kxa_sb[hp][:, j0:j0 + WW], start=False, stop=True)
                nc.tensor.matmul(
                    ps[:, WW:2 * WW], lhsT=ident[:, 0:128],
                    rhs=bias_sb[:, b0 + WW:b0 + 2 * WW], start=True, stop=False)
                nc.tensor.matmul(
                    ps[:, WW:2 * WW], lhsT=q_sb[hp][:, q0:q0 + 128],
                    rhs=kxb_sb[hp][:, j0:j0 + WW], start=False, stop=True)
                ex = ex_pool.tile([128, 2 * WW], bf16, tag="ex")
                den = dn_pool.tile([128, 4], f32, tag="den")
                for hi in range(2):
                    nc.scalar.activation(
                        out=ex[:, hi * WW:(hi + 1) * WW],
                        in_=ps[:, hi * WW:(hi + 1) * WW],
                        func=EXP, accum_out=den[:, hi:hi + 1])
                nc.vector.reciprocal(out=den[:, 2:4], in_=den[:, 0:2])
                at = at_pool.tile([128, 2 * WW], bf16, tag="at")
                for hi in range(2):
                    nc.vector.tensor_scalar_mul(
                        out=at[:, hi * WW:(hi + 1) * WW],
                        in0=ex[:, hi * WW:(hi + 1) * WW],
                        scalar1=den[:, 2 + hi:3 + hi])
                state[(hp, blk)] = at

            def emit_attnout(hp):
                pc = psc_pool.tile([128, NO], f32, tag="psc")
                for blk in range(NBLK):
                    at = state.pop((hp, blk))
                    # pt: [0:128]=hA main^T, [128:256]=hB main^T,
                    #     [0:16, 256:384]=hA tail^T, [0:16, 384:512]=hB tail^T
                    pt = pst_pool.tile([128, 512], bf16, tag="pt")
                    nc.tensor.transpose(pt[:, 0:128], at[:, 0:128], ident[:])
                    nc.tensor.transpose(pt[:, 128:256], at[:, WW:WW + 128],
                                        ident[:])
                    nc.tensor.transpose(pt[0:16, 256:384], at[:, 128:WW],
                                        ident[:])
                    nc.tensor.transpose(pt[0:16, 384:512], at[:, WW + 128:2 * WW],
                                        ident[:])
                    atT = atT_pool.tile([128, 512], bf16, tag="atT")
                    nc.vector.tensor_copy(out=atT[:, 0:256], in_=pt[:, 0:256])
                    nc.vector.tensor_copy(out=atT[0:16, 256:512],
                                          in_=pt[0:16, 256:512])
                    for hi in range(2):
                        h = 2 * hp + hi
                        nc.tensor.matmul(
                            pc[hi * 64:(hi + 1) * 64, blk * 128:(blk + 1) * 128],
                            lhsT=v_sb[blk][:, h * 64:(h + 1) * 64],
                            rhs=atT[:, hi * 128:hi * 128 + 128],
                            start=True, stop=False,
                        )
                        nc.tensor.matmul(
                            pc[hi * 64:(hi + 1) * 64, blk * 128:(blk + 1) * 128],
                            lhsT=v_sb[blk + 1][0:16, h * 64:(h + 1) * 64],
                            rhs=atT[0:16, 256 + hi * 128:256 + hi * 128 + 128],
                            start=False, stop=True,
                        )
                nc.scalar.copy(out=ctxT[hp][:], in_=pc[:])

            for hp in range(8):
                emit_scores(hp, 0)
                emit_scores(hp, 1)
                if 2 * hp + 2 < H:
                    emit_qk(2 * hp + 2)
                emit_scores(hp, 2)
                emit_scores(hp, 3)
                if 2 * hp + 3 < H:
                    emit_qk(2 * hp + 3)
                if hp >= 1:
                    emit_attnout(hp - 1)
            emit_attnout(7)

        # ---- phase O: out_nat[n, o] (ctxT-stationary, k-inner) ---------
        with tc.tile_pool(name="pso", bufs=2, space="PSUM") as pso:
            for t in range(4):
                ps0 = pso.tile([128, 512], f32, tag="pso0")
                ps1 = pso.tile([128, 512], f32, tag="pso1")
                for k in range(KT):
                    nc.tensor.matmul(
                        ps0[:],
                        lhsT=ctxT[k][:, t * 128:(t + 1) * 128],
                        rhs=wpt_t[k][:, 0:512],
                        start=(k == 0), stop=(k == KT - 1),
                    )
                    nc.tensor.matmul(
                        ps1[:],
                        lhsT=ctxT[k][:, t * 128:(t + 1) * 128],
                        rhs=wpt_t[k][:, 512:1024],
                        start=(k == 0), stop=(k == KT - 1),
                    )
                for oh, psx in ((0, ps0), (1, ps1)):
                    ot = out_pool.tile([128, 512], f32, tag="ot")
                    nc.vector.tensor_add(
                        out=ot[:], in0=psx[:],
                        in1=bpb_sb[:, oh * 512:(oh + 1) * 512])
                    nc.sync.dma_start(
                        out=out_e[t * 128:(t + 1) * 128, oh * 512:(oh + 1) * 512],
                        in_=ot[:])

    from concourse import mybir as _mb
    import os as _os
    if _os.environ.get("KERNEL_NO_DEDUP"):
        n = 0
    else:
        n = _dedup_ldweights(nc, _mb)
    _CACHE["ldweights_removed"] = n
    nc.compile()
    return nc


def _get_nc():
    if "nc" not in _CACHE:
        _CACHE["nc"] = _build_nc()
    return _CACHE["nc"]


def _band_width(epoch):
    if epoch is None or epoch >= 50:
        return None
    if epoch < 20:
        return 6
    if epoch < 30:
        return 8
    if epoch < 40:
        return 10
    return 12


def _numpy_ref(x, Wqkv, Wproj, bproj, w):
    """Pure-numpy fallback for band widths this kernel wasn't compiled for."""
    b, n, c = x.shape
    d = c // H
    qkv = np.einsum("bnc,oc->bno", x, Wqkv)
    qkv = qkv.reshape(b, n, H, 3 * d).transpose(0, 2, 1, 3)
    q, k, v = np.split(qkv, 3, axis=-1)
    score = np.einsum("bhid,bhjd->bhij", q, k) * (d ** -0.5)
    if w is not None:
        idx = np.arange(n)
        mask = np.abs(idx[:, None] - idx[None, :]) <= w
        score = np.where(mask[None, None], score, np.float32(-1e9))
    score -= score.max(axis=-1, keepdims=True)
    e = np.exp(score)
    attn = e / e.sum(axis=-1, keepdims=True)
    ctxv = np.einsum("bhij,bhjd->bhid", attn, v)
    ctxv = ctxv.transpose(0, 2, 1, 3).reshape(b, n, c)
    return (np.einsum("bnc,oc->bno", ctxv, Wproj) + bproj).astype(np.float32)


def _prep_in_maps(x, Wqkv, Wproj, bproj):
    import ml_dtypes
    bf = ml_dtypes.bfloat16
    x = np.ascontiguousarray(np.asarray(x, dtype=np.float32))
    Wqkv = np.asarray(Wqkv, dtype=np.float32)
    Wproj = np.asarray(Wproj, dtype=np.float32)
    bproj = np.asarray(bproj, dtype=np.float32)

    # qk weight output-blocks g: even g -> [q_{2hp} | q_{2hp+1}] (prescaled),
    # odd g -> [k_{2hp} | k_{2hp+1}]
    wsplit = Wqkv.reshape(H, 3, D, C)
    wq = wsplit[:, 0] * np.float32(SCALE)                      # [H, D, C]
    wk = wsplit[:, 1]                                          # [H, D, C]
    wv = wsplit[:, 2]                                          # [H, D, C]
    wg = np.empty((H, 128, C), dtype=np.float32)
    wg[0::2] = wq.reshape(8, 128, C)
    wg[1::2] = wk.reshape(8, 128, C)
    # wqkb[g, p, k*128+m] = wg[g, m, k*128+p]: per-g contiguous [128, C]
    # slabs whose col-block k is the k-th contraction tile's lhsT
    wqkb = (wg.transpose(0, 2, 1).reshape(H, KT, 128, 128).transpose(0, 2, 1, 3)
            .reshape(H, 128, C))
    wqkb = np.ascontiguousarray(
        wqkb.transpose(1, 0, 2).reshape(128, H * C)).astype(bf)
    wvt = wv.reshape(H * D, C).T                               # [C, C]
    wvt = np.ascontiguousarray(
        wvt.reshape(KT, 128, C).transpose(1, 0, 2).reshape(128, KT * C)).astype(bf)
    wpt = np.ascontiguousarray(
        Wproj.T.reshape(KT, 128, C).transpose(1, 0, 2)
        .reshape(128, KT * C)).astype(bf)
    bpb = np.ascontiguousarray(
        np.broadcast_to(bproj[None, :], (128, C))).astype(bf)

    # additive score bias per sequence-half s: 0 where in-band and the k
    # column is a real token, else -1e9.  Layout [128, blk*(2*WW)] with the
    # per-block [128, WW] pattern duplicated for the two heads of a pair.
    r = np.arange(128)[:, None]
    jj = np.arange(WW)[None, :]
    band = (jj >= r) & (jj <= r + 2 * HALO)                    # [128, WW]
    biases = []
    for s in (0, 1):
        m = np.full((128, NBLK * 2 * WW), NEG, dtype=np.float32)
        for blk in range(NBLK):
            mloc = blk * 128 + jj                              # local k index
            valid = (mloc >= HALO) if s == 0 else (mloc < NO + HALO)
            bb = np.where(band & valid, 0.0, NEG).astype(np.float32)
            m[:, blk * 2 * WW:blk * 2 * WW + WW] = bb
            m[:, blk * 2 * WW + WW:(blk + 1) * 2 * WW] = bb
        biases.append(m.astype(bf))

    in_maps = []
    for core in range(8):
        b, s = core // 2, core % 2
        xloc = np.zeros((NL, C), dtype=np.float32)
        g0 = s * NO - HALO
        lo, hi = max(0, g0), min(N, g0 + NL)
        xloc[lo - g0:hi - g0] = x[b, lo:hi]
        xtp = np.ascontiguousarray(
            xloc.T.reshape(KT, 128, NL).transpose(1, 0, 2)
            .reshape(128, KT * NL)).astype(bf)
        in_maps.append({
            "xt": xtp,
            "wqkb": wqkb, "wvt": wvt, "wpt": wpt, "bpb": bpb,
            "bias": biases[s],
        })
    return in_maps


def kernel(x, Wqkv, Wproj, bproj, epoch):
    ep = None if epoch is None else int(np.asarray(epoch))
    w = _band_width(ep)
    if w != HALO:
        return _numpy_ref(np.asarray(x, np.float32), np.asarray(Wqkv, np.float32),
                          np.asarray(Wproj, np.float32),
                          np.asarray(bproj, np.float32), w)

    from concourse.bass_utils import run_bass_kernel_spmd

    nc = _get_nc()
    in_maps = _prep_in_maps(x, Wqkv, Wproj, bproj)
    res = run_bass_kernel_spmd(nc, in_maps, core_ids=list(range(8)))
    _CACHE["last_results"] = res

    out = np.empty((B, N, C), dtype=np.float32)
    for core in range(8):
        b, s = core // 2, core % 2
        out[b, s * NO:(s + 1) * NO, :] = res.results[core]["out"]
    return out


# revision 22
# speedup vs baseline: 1.2145x; 1.0272x over previous
"""Banded (sparse) multi-head attention block on 8 TRN2 NeuronCores.

Reference computation (B=4, N=1024, C=1024, H=16, D=64, epoch=25 -> band w=8):
    qkv = x @ Wqkv.T                      [B,N,3C], per-head interleaved split
    q,k,v per head; score = q k^T / sqrt(D); band mask |i-j|<=8; softmax
    ctx = attn @ v; out = ctx @ Wproj.T + bproj

Sharding: the band mask makes attention local, so we shard the sequence:
core = (b, s) with b in 0..3, s in 0..1 owns tokens [s*512, (s+1)*512) of
batch b plus an 8-token halo on each side.  No collectives are needed.

Per-core pipeline (tuned for the tensor engine):
  GEMM-V : v_nat[j, c]  with xt-stationary, dual-PSUM k-inner accumulation
  GEMM-QK: qk^T[c', n]  with w-stationary, dual-PSUM k-inner accumulation
  Attention per (head-pair, 128-row q-block, 144-wide window):
      band mask preloaded into PSUM as additive -1e9 bias (identity matmul),
      scores accumulate on top, exp on ACT with accum_out row-sums (=denoms),
      reciprocal+normalize on DVE, PE-transpose, ctx accumulated per-hp into
      one PSUM bank -> ctxT[hp] slabs [c, n]
  GEMM-O : out_nat[n, o] with ctxT-stationary, dual-PSUM k-inner, bias via
      DVE add during PSUM->SBUF eviction
A post-schedule pass removes back-to-back redundant LDWEIGHTS so paired
matmuls share one stationary load.
"""

import sys

if "/opt/trn_rl_repo" not in sys.path:
    sys.path.insert(0, "/opt/trn_rl_repo")

import numpy as np

B, N, C, H, D = 4, 1024, 1024, 16, 64
NO = 512          # owned tokens per core
HALO = 8
NL = NO + 2 * HALO    # 528 local tokens
WW = 144          # score window width per 128-row q block (128 main + 16 tail)
NBLK = 4          # q blocks of 128 per core
KT = 8            # contraction tiles (1024 / 128)
SCALE = D ** -0.5
NEG = -1.0e9

_CACHE = {}


def _dedup_ldweights(nc, mybir):
    """Remove InstLdweights whose stationary AP + flags match the previous
    ldweights on the PE stream with no intervening control flow.  Sync info
    on a removed duplicate is transferred to the next instruction."""
    removed = 0
    for fn in nc.m.functions:
        for blk in fn.blocks:
            insts = blk.instructions
            last_key = None
            drops = []
            for idx, inst in enumerate(insts):
                tname = type(inst).__name__
                if isinstance(inst, mybir.InstLdweights):
                    key = (
                        repr(inst.ins[0]),
                        getattr(inst, "is_transpose", None),
                        getattr(inst, "perf_mode", None),
                        getattr(inst, "tile_position", None),
                        getattr(inst, "tile_size", None),
                    )
                    if key == last_key:
                        drops.append((idx, inst))
                    else:
                        last_key = key
                elif isinstance(inst, mybir.InstMatmult):
                    pass  # does not change the loaded stationary
                elif "Branch" in tname or "ControlFlow" in tname or "Call" in tname:
                    last_key = None
            for idx, inst in drops:
                if inst.has_wait() or inst.has_update():
                    nxt = insts[idx + 1] if idx + 1 < len(insts) else None
                    if nxt is None:
                        continue
                    nxt.add_sync_dependencies_from(inst)
                insts.remove(inst)
                removed += 1
    return removed


def _build_nc():
    import concourse.bacc as bacc
    import concourse.tile as tile
    from concourse import mybir
    from concourse.masks import make_identity
    from contextlib import ExitStack

    f32 = mybir.dt.float32
    bf16 = mybir.dt.bfloat16
    EXP = mybir.ActivationFunctionType.Exp

    nc = bacc.Bacc(None, target_bir_lowering=False)

    xt_e = nc.declare_dram_parameter("xt", [C, NL], bf16, isOutput=False)
    wqkb_e = nc.declare_dram_parameter("wqkb", [H, 128, C], bf16, isOutput=False)
    wvt_e = nc.declare_dram_parameter("wvt", [C, C], bf16, isOutput=False)
    wpt_e = nc.declare_dram_parameter("wpt", [KT, 128, C], bf16, isOutput=False)
    bpb_e = nc.declare_dram_parameter("bpb", [128, C], bf16, isOutput=False)
    bias_e = nc.declare_dram_parameter("bias", [128, NBLK * 2 * WW], bf16,
                                       isOutput=False)
    out_e = nc.declare_dram_parameter("out", [NO, C], f32, isOutput=True)

    with tile.TileContext(nc) as tc, ExitStack() as ctx:
        const = ctx.enter_context(tc.tile_pool(name="const", bufs=1))
        xts = ctx.enter_context(tc.tile_pool(name="xts", bufs=1))
        wv_pool = ctx.enter_context(tc.tile_pool(name="wvp", bufs=1))
        wqk_pool = ctx.enter_context(tc.tile_pool(name="wqkp", bufs=1))
        wpt_pool = ctx.enter_context(tc.tile_pool(name="wptp", bufs=1))
        qk_pool = ctx.enter_context(tc.tile_pool(name="qksb", bufs=1))
        v_pool = ctx.enter_context(tc.tile_pool(name="vsb", bufs=1))
        ctx_pool = ctx.enter_context(tc.tile_pool(name="ctxsb", bufs=1))
        ex_pool = ctx.enter_context(tc.tile_pool(name="exp", bufs=4))
        at_pool = ctx.enter_context(tc.tile_pool(name="atp", bufs=8))
        atT_pool = ctx.enter_context(tc.tile_pool(name="atTp", bufs=2))
        dn_pool = ctx.enter_context(tc.tile_pool(name="dnp", bufs=4))
        out_pool = ctx.enter_context(tc.tile_pool(name="outp", bufs=3))

        # ---- DMAs (queue order == program order: qk0/qk1 feed first) ----
        wqk_t = []
        for g in range(H):
            t = wqk_pool.tile([128, C], bf16, tag=f"wqk{g}")
            wqk_t.append(t)
        for g in (0, 1):
            nc.sync.dma_start(out=wqk_t[g][:], in_=wqkb_e[g])
        xt_t = []
        for k in range(KT):
            t = xts.tile([128, NL], bf16, tag=f"xt{k}")
            nc.sync.dma_start(out=t[:], in_=xt_e[k * 128:(k + 1) * 128, :])
            xt_t.append(t)
        bias_sb = const.tile([128, NBLK * 2 * WW], bf16, tag="bias")
        nc.sync.dma_start(out=bias_sb[:], in_=bias_e[:])
        wv_t = []
        for k in range(KT):
            t = wv_pool.tile([128, C], bf16, tag=f"wv{k}")
            nc.sync.dma_start(out=t[:], in_=wvt_e[k * 128:(k + 1) * 128, :])
            wv_t.append(t)
        for g in range(2, H):
            nc.sync.dma_start(out=wqk_t[g][:], in_=wqkb_e[g])
        wpt_t = []
        for k in range(KT):
            t = wpt_pool.tile([128, C], bf16, tag=f"wpt{k}")
            nc.sync.dma_start(out=t[:], in_=wpt_e[k])
            wpt_t.append(t)
        bpb_sb = const.tile([128, C], bf16, tag="bpb")
        nc.sync.dma_start(out=bpb_sb[:], in_=bpb_e[:])

        ident = const.tile([128, 128], bf16, tag="ident")
        make_identity(nc, ident[:])

        # persistent activation slabs; kxa = [k_even | 0], kxb = [0 | k_odd]
        # so score matmuls contract the full 128 partitions at base 0.
        q_sb, kxa_sb, kxb_sb = [], [], []
        for hp in range(8):
            tq = qk_pool.tile([128, NL], bf16, tag=f"q{hp}")
            q_sb.append(tq)
            ta = qk_pool.tile([128, NL], bf16, tag=f"kxa{hp}")
            nc.vector.memset(ta[64:128, :], 0.0)
            kxa_sb.append(ta)
            tb = qk_pool.tile([128, NL], bf16, tag=f"kxb{hp}")
            nc.vector.memset(tb[0:64, :], 0.0)
            kxb_sb.append(tb)
        v_sb = []
        for j in range(5):
            t = v_pool.tile([128, C], bf16, tag=f"v{j}")
            v_sb.append(t)
        ctxT = []
        for cb in range(8):
            t = ctx_pool.tile([128, NO], bf16, tag=f"ctx{cb}")
            ctxT.append(t)

        def emit_qk_with(g, ps0, ps1, copy):
            wt = wqk_t[g]
            for k in range(KT):
                nc.tensor.matmul(
                    ps0[:128, 0:264],
                    lhsT=wt[:, k * 128:(k + 1) * 128],
                    rhs=xt_t[k][:, 0:264],
                    start=(k == 0), stop=(k == KT - 1),
                )
                nc.tensor.matmul(
                    ps1[:128, 0:264],
                    lhsT=wt[:, k * 128:(k + 1) * 128],
                    rhs=xt_t[k][:, 264:528],
                    start=(k == 0), stop=(k == KT - 1),
                )
            copy(g, ps0, ps1)

        def qk_copies(g, ps0, ps1):
            if g % 2 == 0:
                nc.scalar.copy(out=q_sb[g // 2][:, 0:264], in_=ps0[:128, 0:264])
                nc.vector.tensor_copy(out=q_sb[g // 2][:, 264:528],
                                      in_=ps1[:128, 0:264])
            else:
                nc.scalar.copy(
                    out=kxa_sb[g // 2][0:64, 0:264], in_=ps0[0:64, 0:264])
                nc.vector.tensor_copy(
                    out=kxb_sb[g // 2][64:128, 0:264], in_=ps0[64:128, 0:264])
                nc.scalar.copy(
                    out=kxa_sb[g // 2][0:64, 264:528], in_=ps1[0:64, 0:264])
                nc.vector.tensor_copy(
                    out=kxb_sb[g // 2][64:128, 264:528], in_=ps1[64:128, 0:264])

        # ---- phase V (prefixed by qk0/qk1 warm-up while wv streams) ----
        with tc.tile_pool(name="psv", bufs=2, space="PSUM") as psv:
            for g in (0, 1):
                p0 = psv.tile([128, 512], f32, tag="psv0")
                p1 = psv.tile([128, 512], f32, tag="psv1")
                emit_qk_with(g, p0, p1, qk_copies)
            for jb in range(5):
                pj = 128 if jb < 4 else 16
                ps0 = psv.tile([128, 512], f32, tag="psv0")
                ps1 = psv.tile([128, 512], f32, tag="psv1")
                for k in range(KT):
                    nc.tensor.matmul(
                        ps0[:pj, :],
                        lhsT=xt_t[k][:, jb * 128:jb * 128 + pj],
                        rhs=wv_t[k][:, 0:512],
                        start=(k == 0), stop=(k == KT - 1),
                    )
                    nc.tensor.matmul(
                        ps1[:pj, :],
                        lhsT=xt_t[k][:, jb * 128:jb * 128 + pj],
                        rhs=wv_t[k][:, 512:1024],
                        start=(k == 0), stop=(k == KT - 1),
                    )
                nc.scalar.copy(out=v_sb[jb][:pj, 0:512], in_=ps0[:pj, :])
                nc.scalar.copy(out=v_sb[jb][:pj, 512:1024], in_=ps1[:pj, :])

        # ---- main: QK GEMM interleaved with attention ------------------
        with tc.tile_pool(name="psqk", bufs=2, space="PSUM") as psqk, \
             tc.tile_pool(name="pss", bufs=2, space="PSUM") as pss_pool, \
             tc.tile_pool(name="pst", bufs=1, space="PSUM") as pst_pool, \
             tc.tile_pool(name="psc", bufs=1, space="PSUM") as psc_pool:
            state = {}

            def emit_qk(g):
                ps0 = psqk.tile([128, 264], f32, tag="psqk0")
                ps1 = psqk.tile([128, 264], f32, tag="psqk1")
                emit_qk_with(g, ps0, ps1, qk_copies)

            def emit_scores(hp, blk):
                j0 = blk * 128
                q0 = HALO + blk * 128
                b0 = blk * 2 * WW
                ps = pss_pool.tile([128, 2 * WW], f32, tag="pss")
                nc.tensor.matmul(
                    ps[:, 0:WW], lhsT=ident[:, 0:128],
                    rhs=bias_sb[:, b0:b0 + WW], start=True, stop=False)
                nc.tensor.matmul(
                    ps[:, 0:WW], lhsT=q_sb[hp][:, q0:q0 + 128],
                    rhs=kxa_sb[hp][:, j0:j0 + WW], start=False, stop=True)
                nc.tensor.matmul(
                    ps[:, WW:2 * WW], lhsT=ident[:, 0:128],
                    rhs=bias_sb[:, b0 + WW:b0 + 2 * WW], start=True, stop=False)
                nc.tensor.matmul(
                    ps[:, WW:2 * WW], lhsT=q_sb[hp][:, q0:q0 + 128],
                    rhs=kxb_sb[hp][:, j0:j0 + WW], start=False, stop=True)
                ex = ex_pool.tile([128, 2 * WW], bf16, tag="ex")
                den = dn_pool.tile([128, 4], f32, tag="den")
                for hi in range(2):
                    nc.scalar.activation(
                        out=ex[:, hi * WW:(hi + 1) * WW],
                        in_=ps[:, hi * WW:(hi + 1) * WW],
                        func=EXP, accum_out=den[:, hi:hi + 1])
                nc.vector.reciprocal(out=den[:, 2:4], in_=den[:, 0:2])
                at = at_pool.tile([128, 2 * WW], bf16, tag="at")
                for hi in range(2):
                    nc.vector.tensor_scalar_mul(
                        out=at[:, hi * WW:(hi + 1) * WW],
                        in0=ex[:, hi * WW:(hi + 1) * WW],
                        scalar1=den[:, 2 + hi:3 + hi])
                state[(hp, blk)] = at

            def emit_attnout(hp):
                pc = psc_pool.tile([128, NO], f32, tag="psc")
                for blk in range(NBLK):
                    at = state.pop((hp, blk))
                    # pt: [0:128]=hA main^T, [128:256]=hB main^T,
                    #     [0:16, 256:384]=hA tail^T, [0:16, 384:512]=hB tail^T
                    pt = pst_pool.tile([128, 512], bf16, tag="pt")
                    nc.tensor.transpose(pt[:, 0:128], at[:, 0:128], ident[:])
                    nc.tensor.transpose(pt[:, 128:256], at[:, WW:WW + 128],
                                        ident[:])
                    nc.tensor.transpose(pt[0:16, 256:384], at[:, 128:WW],
                                        ident[:])
                    nc.tensor.transpose(pt[0:16, 384:512], at[:, WW + 128:2 * WW],
                                        ident[:])
                    atT = atT_pool.tile([128, 512], bf16, tag="atT")
                    nc.vector.tensor_copy(out=atT[:, 0:256], in_=pt[:, 0:256])
                    nc.vector.tensor_copy(out=atT[0:16, 256:512],
                                          in_=pt[0:16, 256:512])
                    for hi in range(2):
                        h = 2 * hp + hi
                        nc.tensor.matmul(
                            pc[hi * 64:(hi + 1) * 64, blk * 128:(blk + 1) * 128],
                            lhsT=v_sb[blk][:, h * 64:(h + 1) * 64],
                            rhs=atT[:, hi * 128:hi * 128 + 128],
                            start=True, stop=False,
                        )
                        nc.tensor.matmul(
                            pc[hi * 64:(hi + 1) * 64, blk * 128:(blk + 1) * 128],
                            lhsT=v_sb[blk + 1][0:16, h * 64:(h + 1) * 64],
                            rhs=atT[0:16, 256 + hi * 128:256 + hi * 128 + 128],
                            start=False, stop=True,
                        )
                nc.scalar.copy(out=ctxT[hp][:], in_=pc[:])

            for hp in range(8):
                emit_scores(hp, 0)
                emit_scores(hp, 1)
                if 2 * hp + 2 < H:
                    emit_qk(2 * hp + 2)
                emit_scores(hp, 2)
                emit_scores(hp, 3)
                if 2 * hp + 3 < H:
                    emit_qk(2 * hp + 3)
                if hp >= 1:
                    emit_attnout(hp - 1)
            emit_attnout(7)

        # ---- phase O: out_nat[n, o] (ctxT-stationary, k-inner) ---------
        with tc.tile_pool(name="pso", bufs=2, space="PSUM") as pso:
            for t in range(4):
                ps0 = pso.tile([128, 512], f32, tag="pso0")
                ps1 = pso.tile([128, 512], f32, tag="pso1")
                for k in range(KT):
                    nc.tensor.matmul(
                        ps0[:],
                        lhsT=ctxT[k][:, t * 128:(t + 1) * 128],
                        rhs=wpt_t[k][:, 0:512],
                        start=(k == 0), stop=(k == KT - 1),
                    )
                    nc.tensor.matmul(
                        ps1[:],
                        lhsT=ctxT[k][:, t * 128:(t + 1) * 128],
                        rhs=wpt_t[k][:, 512:1024],
                        start=(k == 0), stop=(k == KT - 1),
                    )
                for oh, psx in ((0, ps0), (1, ps1)):
                    ot = out_pool.tile([128, 512], f32, tag="ot")
                    nc.vector.tensor_add(
                        out=ot[:], in0=psx[:],
                        in1=bpb_sb[:, oh * 512:(oh + 1) * 512])
                    nc.sync.dma_start(
                        out=out_e[t * 128:(t + 1) * 128, oh * 512:(oh + 1) * 512],
                        in_=ot[:])

    from concourse import mybir as _mb
    import os as _os
    if _os.environ.get("KERNEL_NO_DEDUP"):
        n = 0
    else:
        n = _dedup_ldweights(nc, _mb)
    _CACHE["ldweights_removed"] = n
    nc.compile()
    return nc


def _get_nc():
    if "nc" not in _CACHE:
        _CACHE["nc"] = _build_nc()
    return _CACHE["nc"]


def _band_width(epoch):
    if epoch is None or epoch >= 50:
        return None
    if epoch < 20:
        return 6
    if epoch < 30:
        return 8
    if epoch < 40:
        return 10
    return 12


def _numpy_ref(x, Wqkv, Wproj, bproj, w):
    """Pure-numpy fallback for band widths this kernel wasn't compiled for."""
    b, n, c = x.shape
    d = c // H
    qkv = np.einsum("bnc,oc->bno", x, Wqkv)
    qkv = qkv.reshape(b, n, H, 3 * d).transpose(0, 2, 1, 3)
    q, k, v = np.split(qkv, 3, axis=-1)
    score = np.einsum("bhid,bhjd->bhij", q, k) * (d ** -0.5)
    if w is not None:
        idx = np.arange(n)
        mask = np.abs(idx[:, None] - idx[None, :]) <= w
        score = np.where(mask[None, None], score, np.float32(-1e9))
    score -= score.max(axis=-1, keepdims=True)
    e = np.exp(score)
    attn = e / e.sum(axis=-1, keepdims=True)
    ctxv = np.einsum("bhij,bhjd->bhid", attn, v)
    ctxv = ctxv.transpose(0, 2, 1, 3).reshape(b, n, c)
    return (np.einsum("bnc,oc->bno", ctxv, Wproj) + bproj).astype(np.float32)


def _prep_in_maps(x, Wqkv, Wproj, bproj):
    import ml_dtypes
    bf = ml_dtypes.bfloat16
    x = np.ascontiguousarray(np.asarray(x, dtype=np.float32))
    Wqkv = np.asarray(Wqkv, dtype=np.float32)
    Wproj = np.asarray(Wproj, dtype=np.float32)
    bproj = np.asarray(bproj, dtype=np.float32)

    # qk weight output-blocks g: even g -> [q_{2hp} | q_{2hp+1}] (prescaled),
    # odd g -> [k_{2hp} | k_{2hp+1}]
    wsplit = Wqkv.reshape(H, 3, D, C)
    wq = wsplit[:, 0] * np.float32(SCALE)                      # [H, D, C]
    wk = wsplit[:, 1]                                          # [H, D, C]
    wv = wsplit[:, 2]                                          # [H, D, C]
    wg = np.empty((H, 128, C), dtype=np.float32)
    wg[0::2] = wq.reshape(8, 128, C)
    wg[1::2] = wk.reshape(8, 128, C)
    # wqkb[g, p, k*128+m] = wg[g, m, k*128+p]: per-g contiguous [128, C]
    # slabs whose col-block k is the k-th contraction tile's lhsT
    wqkb = np.ascontiguousarray(
        wg.transpose(0, 2, 1).reshape(H, KT, 128, 128).transpose(0, 2, 1, 3)
        .reshape(H, 128, C)).astype(bf)
    wvt = np.ascontiguousarray(wv.reshape(H * D, C).T).astype(bf)  # [C, C]
    wpt = np.ascontiguousarray(Wproj.T.reshape(KT, 128, C)).astype(bf)
    bpb = np.ascontiguousarray(
        np.broadcast_to(bproj[None, :], (128, C))).astype(bf)

    # additive score bias per sequence-half s: 0 where in-band and the k
    # column is a real token, else -1e9.  Layout [128, blk*(2*WW)] with the
    # per-block [128, WW] pattern duplicated for the two heads of a pair.
    r = np.arange(128)[:, None]
    jj = np.arange(WW)[None, :]
    band = (jj >= r) & (jj <= r + 2 * HALO)                    # [128, WW]
    biases = []
    for s in (0, 1):
        m = np.full((128, NBLK * 2 * WW), NEG, dtype=np.float32)
        for blk in range(NBLK):
            mloc = blk * 128 + jj                              # local k index
            valid = (mloc >= HALO) if s == 0 else (mloc < NO + HALO)
            bb = np.where(band & valid, 0.0, NEG).astype(np.float32)
            m[:, blk * 2 * WW:blk * 2 * WW + WW] = bb
            m[:, blk * 2 * WW + WW:(blk + 1) * 2 * WW] = bb
        biases.append(m.astype(bf))

    in_maps = []
    for core in range(8):
        b, s = core // 2, core % 2
        xloc = np.zeros((NL, C), dtype=np.float32)
        g0 = s * NO - HALO
        lo, hi = max(0, g0), min(N, g0 + NL)
        xloc[lo - g0:hi - g0] = x[b, lo:hi]
        in_maps.append({
            "xt": np.ascontiguousarray(xloc.T).astype(bf),
            "wqkb": wqkb, "wvt": wvt, "wpt": wpt, "bpb": bpb,
            "bias": biases[s],
        })
    return in_maps


def kernel(x, Wqkv, Wproj, bproj, epoch):
    ep = None if epoch is None else int(np.asarray(epoch))
    w = _band_width(ep)
    if w != HALO:
        return _numpy_ref(np.asarray(x, np.float32), np.asarray(Wqkv, np.float32),
                          np.asarray(Wproj, np.float32),
                          np.asarray(bproj, np.float32), w)

    from concourse.bass_utils import run_bass_kernel_spmd

    nc = _get_nc()
    in_maps = _prep_in_maps(x, Wqkv, Wproj, bproj)
    res = run_bass_kernel_spmd(nc, in_maps, core_ids=list(range(8)))
    _CACHE["last_results"] = res

    out = np.empty((B, N, C), dtype=np.float32)
    for core in range(8):
        b, s = core // 2, core % 2
        out[b, s * NO:(s + 1) * NO, :] = res.results[core]["out"]
    return out
